# revision 1
# baseline (speedup 1.0000x reference)
"""Low-rank Mahalanobis distance kernel for 8x TRN2 NeuronCores.

Full op: d2[i,j] = max(0, ||L(x_i - y_j)||^2) for x,y [8192,1024], L [128,1024].

Strategy:
  - Host precomputes the cheap projections xL = x@L.T, yL = y@L.T (~2% of
    total FLOPs) plus row norms, and lays everything out in the layouts the
    PE wants (rank on partitions).
  - Rows of x are sharded 8 ways; each core computes a [1024, 8192] slice of
    the output: for each [128,512] tile, one K=128 matmul gives the cross
    term c = xL@yL.T; a K=1 rank-1 matmul accumulates -yn/2 into the same
    PSUM bank; ScalarE then writes Relu(-2*psum + xn) straight to SBUF, and
    each finished [128, 8192] strip goes to HBM as one 4MB DMA.

  d2 = -2*(c - yn/2) + xn = xn + yn - 2c, clamped by the Relu.
"""

import sys

sys.path.insert(0, "/opt/trn_rl_repo")

import numpy as np

N = 8192  # rows of x == output rows
M = 8192  # rows of y == output cols
DIM = 1024
RANK = 128
N_CORES = 8
ROWS_PER_CORE = N // N_CORES  # 1024
IB = ROWS_PER_CORE // 128  # 8 i-blocks per core
JW = 512  # moving free dim per matmul (one PSUM bank of f32)
JT = M // JW  # 16 j-tiles per strip

_CACHE = {}


def _build_nc():
    from contextlib import ExitStack

    import concourse.bacc as bacc
    import concourse.mybir as mybir
    import concourse.tile as tile

    dt = mybir.dt
    nc = bacc.Bacc("TRN2", target_bir_lowering=False, debug=False)

    xlt = nc.dram_tensor("xlt", [RANK, ROWS_PER_CORE], dt.float32, kind="ExternalInput").ap()
    ylt = nc.dram_tensor("ylt", [RANK, M], dt.float32, kind="ExternalInput").ap()
    xn = nc.dram_tensor("xn", [128, IB], dt.float32, kind="ExternalInput").ap()
    ynm2 = nc.dram_tensor("ynm2", [1, M], dt.float32, kind="ExternalInput").ap()
    out = nc.dram_tensor("out", [ROWS_PER_CORE, M], dt.float32, kind="ExternalOutput").ap()

    with tile.TileContext(nc) as tc, ExitStack() as ctx:
        consts = ctx.enter_context(tc.tile_pool(name="consts", bufs=1))
        strips = ctx.enter_context(tc.tile_pool(name="strips", bufs=2))
        psum = ctx.enter_context(tc.tile_pool(name="psum", bufs=8, space="PSUM"))

        xlt_sb = consts.tile([RANK, ROWS_PER_CORE], dt.float32)
        nc.sync.dma_start(xlt_sb[:], xlt[:])
        ylt_sb = consts.tile([RANK, M], dt.float32)
        nc.sync.dma_start(ylt_sb[:], ylt[:])
        xn_sb = consts.tile([128, IB], dt.float32)
        nc.sync.dma_start(xn_sb[:], xn[:])
        yn_sb = consts.tile([1, M], dt.float32)
        nc.sync.dma_start(yn_sb[:], ynm2[:])
        ones_sb = consts.tile([1, 128], dt.float32)
        nc.vector.memset(ones_sb[:], 1.0)

        relu = mybir.ActivationFunctionType.Relu
        for ib in range(IB):
            strip = strips.tile([128, M], dt.float32, tag="strip")
            for jt in range(JT):
                pt = psum.tile([128, JW], dt.float32, tag="pt")
                nc.tensor.matmul(
                    pt[:],
                    lhsT=xlt_sb[:, ib * 128 : (ib + 1) * 128],
                    rhs=ylt_sb[:, jt * JW : (jt + 1) * JW],
                    start=True,
                    stop=False,
                )
                nc.tensor.matmul(
                    pt[:],
                    lhsT=ones_sb[:],
                    rhs=yn_sb[:, jt * JW : (jt + 1) * JW],
                    start=False,
                    stop=True,
                )
                nc.scalar.activation(
                    strip[:, jt * JW : (jt + 1) * JW],
                    pt[:],
                    relu,
                    bias=xn_sb[:, ib : ib + 1],
                    scale=-2.0,
                )
            nc.sync.dma_start(out[ib * 128 : (ib + 1) * 128, :], strip[:])

    nc.compile()
    return nc


def _prepare_in_maps(x, y, L):
    x = np.ascontiguousarray(x, dtype=np.float32)
    y = np.ascontiguousarray(y, dtype=np.float32)
    L = np.ascontiguousarray(L, dtype=np.float32)

    xL = x @ L.T  # [N, RANK]
    yL = y @ L.T  # [M, RANK]
    xn = np.einsum("ij,ij->i", xL, xL).astype(np.float32)  # [N]
    yn = np.einsum("ij,ij->i", yL, yL).astype(np.float32)  # [M]

    xLT = np.ascontiguousarray(xL.T)  # [RANK, N]
    yLT = np.ascontiguousarray(yL.T)  # [RANK, M]
    ynm2 = np.ascontiguousarray((-0.5 * yn).astype(np.float32).reshape(1, M))

    in_maps = []
    for c in range(N_CORES):
        r0 = c * ROWS_PER_CORE
        r1 = r0 + ROWS_PER_CORE
        # xn in [128 partitions, IB] column layout: col b holds xn of i-block b
        xn_cols = np.ascontiguousarray(xn[r0:r1].reshape(IB, 128).T)
        in_maps.append(
            {
                "xlt": np.ascontiguousarray(xLT[:, r0:r1]),
                "ylt": yLT,
                "xn": xn_cols,
                "ynm2": ynm2,
            }
        )
    return in_maps


def run_sharded(x, y, L, trace=False, trace_cores=None):
    """Run the device kernel; returns (full_output, BassKernelResults)."""
    from concourse.bass_utils import run_bass_kernel_spmd

    if "nc" not in _CACHE:
        _CACHE["nc"] = _build_nc()
    nc = _CACHE["nc"]

    in_maps = _prepare_in_maps(x, y, L)
    res = run_bass_kernel_spmd(
        nc,
        in_maps,
        list(range(N_CORES)),
        trace=trace,
        trace_cores=trace_cores,
    )
    full = np.concatenate([r["out"] for r in res.results], axis=0)
    return full, res


def kernel(x, y, L):
    full, _ = run_sharded(x, y, L)
    return full


# revision 2
# speedup vs baseline: 3.1889x; 3.1889x over previous
"""Low-rank Mahalanobis distance kernel for 8x TRN2 NeuronCores.

Full op: d2[i,j] = max(0, ||L(x_i - y_j)||^2) for x,y [8192,1024], L [128,1024].

Strategy:
  - Host precomputes the cheap projections xL = x@L.T, yL = y@L.T (~2% of
    total FLOPs) plus row norms, and lays everything out in the layouts the
    PE wants (rank on partitions). The -2 of the cross term is folded into
    the x projection on the host.
  - Rows of x are sharded 8 ways; each core computes a [1024, 8192] slice of
    the output. Per [128,1024] PSUM tile (2 banks): two K=128 bf16 matmuls
    give -2*cross; two K=2 rank-1 matmuls accumulate yn_j (bf16 hi+lo split
    so the add is ~f32-accurate); the epilogue Relu(psum + xn_i) runs on
    ScalarE (activation, per-partition f32 bias) and VectorE (tensor_scalar
    add+max) on alternating tiles, writing into a [128, 8192] SBUF strip
    that ships to HBM as one 4MB DMA.

  fp32 matmuls are avoided deliberately: they lower to HI/LO instruction
  pairs and throttle the PE clock to half rate (measured 500us PE time vs
  ~55us for bf16).
"""

import sys

sys.path.insert(0, "/opt/trn_rl_repo")

import ml_dtypes
import numpy as np

N = 8192  # rows of x == output rows
M = 8192  # rows of y == output cols
DIM = 1024
RANK = 128
N_CORES = 8
ROWS_PER_CORE = N // N_CORES  # 1024
IB = ROWS_PER_CORE // 128  # 8 i-blocks per core
JW = 512  # moving free dim per matmul (one PSUM bank of f32)
PTW = 1024  # psum tile width (2 banks) -> one epilogue op per 1024 cols
JT = M // PTW  # 8 psum tiles per strip

BF16 = ml_dtypes.bfloat16

_CACHE = {}


def _build_nc():
    from contextlib import ExitStack

    import concourse.bacc as bacc
    import concourse.mybir as mybir
    import concourse.tile as tile

    dt = mybir.dt
    nc = bacc.Bacc("TRN2", target_bir_lowering=False, debug=False)

    xlt = nc.dram_tensor("xlt", [RANK, ROWS_PER_CORE], dt.bfloat16, kind="ExternalInput").ap()
    ylt = nc.dram_tensor("ylt", [RANK, M], dt.bfloat16, kind="ExternalInput").ap()
    xn = nc.dram_tensor("xn", [128, IB], dt.float32, kind="ExternalInput").ap()
    ynp = nc.dram_tensor("ynp", [2, M], dt.bfloat16, kind="ExternalInput").ap()
    out = nc.dram_tensor("out", [ROWS_PER_CORE, M], dt.float32, kind="ExternalOutput").ap()

    with tile.TileContext(nc) as tc, ExitStack() as ctx:
        consts = ctx.enter_context(tc.tile_pool(name="consts", bufs=1))
        strips = ctx.enter_context(tc.tile_pool(name="strips", bufs=2))
        psum = ctx.enter_context(tc.tile_pool(name="psum", bufs=4, space="PSUM"))

        xlt_sb = consts.tile([RANK, ROWS_PER_CORE], dt.bfloat16)
        nc.sync.dma_start(xlt_sb[:], xlt[:])
        ylt_sb = consts.tile([RANK, M], dt.bfloat16)
        nc.sync.dma_start(ylt_sb[:], ylt[:])
        xn_sb = consts.tile([128, IB], dt.float32)
        nc.sync.dma_start(xn_sb[:], xn[:])
        yn_sb = consts.tile([2, M], dt.bfloat16)
        nc.sync.dma_start(yn_sb[:], ynp[:])
        ones_sb = consts.tile([2, 128], dt.bfloat16)
        nc.vector.memset(ones_sb[:], 1.0)

        relu = mybir.ActivationFunctionType.Relu
        alu = mybir.AluOpType
        for ib in range(IB):
            strip = strips.tile([128, M], dt.float32, tag="strip")
            xlt_blk = xlt_sb[:, ib * 128 : (ib + 1) * 128]
            xn_col = xn_sb[:, ib : ib + 1]
            for jt in range(JT):
                pt = psum.tile([128, PTW], dt.float32, tag="pt")
                for h in range(PTW // JW):
                    j0 = jt * PTW + h * JW
                    nc.tensor.matmul(
                        pt[:, h * JW : (h + 1) * JW],
                        lhsT=xlt_blk,
                        rhs=ylt_sb[:, j0 : j0 + JW],
                        start=True,
                        stop=False,
                    )
                    nc.tensor.matmul(
                        pt[:, h * JW : (h + 1) * JW],
                        lhsT=ones_sb[:],
                        rhs=yn_sb[:, j0 : j0 + JW],
                        start=False,
                        stop=True,
                    )
                dst = strip[:, jt * PTW : (jt + 1) * PTW]
                if jt % 2 == 0:
                    nc.scalar.activation(dst, pt[:], relu, bias=xn_col, scale=1.0)
                else:
                    nc.vector.tensor_scalar(dst, pt[:], xn_col, 0.0, alu.add, alu.max)
            nc.sync.dma_start(out[ib * 128 : (ib + 1) * 128, :], strip[:])

    nc.compile()
    return nc


def _prepare_in_maps(x, y, L):
    x = np.ascontiguousarray(x, dtype=np.float32)
    y = np.ascontiguousarray(y, dtype=np.float32)
    L = np.ascontiguousarray(L, dtype=np.float32)

    xL = x @ L.T  # [N, RANK]
    yL = y @ L.T  # [M, RANK]
    xn = np.einsum("ij,ij->i", xL, xL).astype(np.float32)  # [N]
    yn = np.einsum("ij,ij->i", yL, yL).astype(np.float32)  # [M]

    xLT = np.ascontiguousarray((-2.0 * xL).T.astype(BF16))  # [RANK, N]
    yLT = np.ascontiguousarray(yL.T.astype(BF16))  # [RANK, M]
    # yn as bf16 hi + residual-lo rows so the rank-1 plane add is near-f32
    yn_hi = yn.astype(BF16)
    yn_lo = (yn - yn_hi.astype(np.float32)).astype(BF16)
    ynp = np.ascontiguousarray(np.stack([yn_hi, yn_lo], axis=0))  # [2, M] bf16

    in_maps = []
    for c in range(N_CORES):
        r0 = c * ROWS_PER_CORE
        r1 = r0 + ROWS_PER_CORE
        # xn in [128 partitions, IB] column layout: col b holds xn of i-block b
        xn_cols = np.ascontiguousarray(xn[r0:r1].reshape(IB, 128).T)
        in_maps.append(
            {
                "xlt": np.ascontiguousarray(xLT[:, r0:r1]),
                "ylt": yLT,
                "xn": xn_cols,
                "ynp": ynp,
            }
        )
    return in_maps


def run_sharded(x, y, L, trace=False, trace_cores=None):
    """Run the device kernel; returns (full_output, BassKernelResults)."""
    from concourse.bass_utils import run_bass_kernel_spmd

    if "nc" not in _CACHE:
        _CACHE["nc"] = _build_nc()
    nc = _CACHE["nc"]

    in_maps = _prepare_in_maps(x, y, L)
    res = run_bass_kernel_spmd(
        nc,
        in_maps,
        list(range(N_CORES)),
        trace=trace,
        trace_cores=trace_cores,
    )
    full = np.concatenate([r["out"] for r in res.results], axis=0)
    return full, res


def kernel(x, y, L):
    full, _ = run_sharded(x, y, L)
    return full


# revision 5
# speedup vs baseline: 3.7874x; 1.1877x over previous
"""Low-rank Mahalanobis distance kernel for 8x TRN2 NeuronCores.

Full op: d2[i,j] = max(0, ||L(x_i - y_j)||^2) for x,y [8192,1024], L [128,1024].

Strategy:
  - Host precomputes the cheap projections xL = x@L.T, yL = y@L.T (~2% of
    total FLOPs) plus row norms, and lays everything out in the layouts the
    PE wants (rank on partitions). The -2 of the cross term is folded into
    the x projection on the host.
  - Rows of x are sharded 8 ways; each core computes a [1024, 8192] slice of
    the output. Per [128,1024] PSUM tile (2 banks): two K=128 bf16 matmuls
    give -2*cross; two K=2 rank-1 matmuls accumulate yn_j (bf16 hi+lo split
    so the add is ~f32-accurate); the epilogue Relu(psum + xn_i) runs on
    ScalarE (activation, per-partition f32 bias) and VectorE (tensor_scalar
    add+max) on alternating tiles, writing into a [128, 8192] SBUF strip
    that ships to HBM as one 4MB DMA.

  fp32 matmuls are avoided deliberately: they lower to HI/LO instruction
  pairs and throttle the PE clock to half rate (measured 500us PE time vs
  ~55us for bf16).
"""

import sys

sys.path.insert(0, "/opt/trn_rl_repo")

import ml_dtypes
import numpy as np

N = 8192  # rows of x == output rows
M = 8192  # rows of y == output cols
DIM = 1024
RANK = 128
N_CORES = 8
ROWS_PER_CORE = N // N_CORES  # 1024
IB = ROWS_PER_CORE // 128  # 8 i-blocks per core
JW = 512  # moving free dim per matmul (one PSUM bank of f32)
PTW = 1024  # psum tile width (2 banks) -> one epilogue op per 1024 cols
JT = M // PTW  # 8 psum tiles per strip

BF16 = ml_dtypes.bfloat16

_CACHE = {}


def _build_nc():
    from contextlib import ExitStack

    import concourse.bacc as bacc
    import concourse.mybir as mybir
    import concourse.tile as tile

    dt = mybir.dt
    nc = bacc.Bacc("TRN2", target_bir_lowering=False, debug=False)

    xlt = nc.dram_tensor("xlt", [RANK, ROWS_PER_CORE], dt.bfloat16, kind="ExternalInput").ap()
    ylt = nc.dram_tensor("ylt", [RANK, M], dt.bfloat16, kind="ExternalInput").ap()
    xn = nc.dram_tensor("xn", [128, IB], dt.float32, kind="ExternalInput").ap()
    ynp = nc.dram_tensor("ynp", [2, M], dt.bfloat16, kind="ExternalInput").ap()
    out = nc.dram_tensor("out", [ROWS_PER_CORE, M], dt.float32, kind="ExternalOutput").ap()

    with tile.TileContext(nc) as tc, ExitStack() as ctx:
        consts = ctx.enter_context(tc.tile_pool(name="consts", bufs=1))
        strips = ctx.enter_context(tc.tile_pool(name="strips", bufs=2))
        psum = ctx.enter_context(tc.tile_pool(name="psum", bufs=1, space="PSUM"))

        xlt_sb = consts.tile([RANK, ROWS_PER_CORE], dt.bfloat16)
        nc.sync.dma_start(xlt_sb[:], xlt[:])
        ylt_sb = consts.tile([RANK, M], dt.bfloat16)
        nc.sync.dma_start(ylt_sb[:], ylt[:])
        xn_sb = consts.tile([128, IB], dt.float32)
        nc.sync.dma_start(xn_sb[:], xn[:])
        yn_sb = consts.tile([2, M], dt.bfloat16)
        nc.sync.dma_start(yn_sb[:], ynp[:])
        ones_sb = consts.tile([2, 128], dt.bfloat16)
        nc.vector.memset(ones_sb[:], 1.0)

        relu = mybir.ActivationFunctionType.Relu
        alu = mybir.AluOpType
        GRP = 4  # psum tiles per matmul batch (4 x 2 banks = all of PSUM)
        for ib in range(IB):
            strip = strips.tile([128, M], dt.float32, tag="strip")
            xlt_blk = xlt_sb[:, ib * 128 : (ib + 1) * 128]
            xn_col = xn_sb[:, ib : ib + 1]
            for g in range(JT // GRP):
                pts = [
                    psum.tile([128, PTW], dt.float32, tag=f"pt{k}", name=f"pt{k}")
                    for k in range(GRP)
                ]
                # all cross matmuls back-to-back: stationary xlt_blk loads once
                for k in range(GRP):
                    jt = g * GRP + k
                    for h in range(PTW // JW):
                        j0 = jt * PTW + h * JW
                        nc.tensor.matmul(
                            pts[k][:, h * JW : (h + 1) * JW],
                            lhsT=xlt_blk,
                            rhs=ylt_sb[:, j0 : j0 + JW],
                            start=True,
                            stop=False,
                        )
                # then all yn-plane matmuls: stationary ones_sb loads once
                for k in range(GRP):
                    jt = g * GRP + k
                    for h in range(PTW // JW):
                        j0 = jt * PTW + h * JW
                        nc.tensor.matmul(
                            pts[k][:, h * JW : (h + 1) * JW],
                            lhsT=ones_sb[:],
                            rhs=yn_sb[:, j0 : j0 + JW],
                            start=False,
                            stop=True,
                        )
                for k in range(GRP):
                    jt = g * GRP + k
                    dst = strip[:, jt * PTW : (jt + 1) * PTW]
                    if k % 2 == 0:
                        nc.scalar.activation(dst, pts[k][:], relu, bias=xn_col, scale=1.0)
                    else:
                        nc.vector.tensor_scalar(dst, pts[k][:], xn_col, 0.0, alu.add, alu.max)
            nc.sync.dma_start(out[ib * 128 : (ib + 1) * 128, :], strip[:])

    nc.compile()
    return nc


def _prepare_in_maps(x, y, L):
    x = np.ascontiguousarray(x, dtype=np.float32)
    y = np.ascontiguousarray(y, dtype=np.float32)
    L = np.ascontiguousarray(L, dtype=np.float32)

    xL = x @ L.T  # [N, RANK]
    yL = y @ L.T  # [M, RANK]
    xn = np.einsum("ij,ij->i", xL, xL).astype(np.float32)  # [N]
    yn = np.einsum("ij,ij->i", yL, yL).astype(np.float32)  # [M]

    xLT = np.ascontiguousarray((-2.0 * xL).T.astype(BF16))  # [RANK, N]
    yLT = np.ascontiguousarray(yL.T.astype(BF16))  # [RANK, M]
    # yn as bf16 hi + residual-lo rows so the rank-1 plane add is near-f32
    yn_hi = yn.astype(BF16)
    yn_lo = (yn - yn_hi.astype(np.float32)).astype(BF16)
    ynp = np.ascontiguousarray(np.stack([yn_hi, yn_lo], axis=0))  # [2, M] bf16

    in_maps = []
    for c in range(N_CORES):
        r0 = c * ROWS_PER_CORE
        r1 = r0 + ROWS_PER_CORE
        # xn in [128 partitions, IB] column layout: col b holds xn of i-block b
        xn_cols = np.ascontiguousarray(xn[r0:r1].reshape(IB, 128).T)
        in_maps.append(
            {
                "xlt": np.ascontiguousarray(xLT[:, r0:r1]),
                "ylt": yLT,
                "xn": xn_cols,
                "ynp": ynp,
            }
        )
    return in_maps


def run_sharded(x, y, L, trace=False, trace_cores=None):
    """Run the device kernel; returns (full_output, BassKernelResults)."""
    from concourse.bass_utils import run_bass_kernel_spmd

    if "nc" not in _CACHE:
        _CACHE["nc"] = _build_nc()
    nc = _CACHE["nc"]

    in_maps = _prepare_in_maps(x, y, L)
    res = run_bass_kernel_spmd(
        nc,
        in_maps,
        list(range(N_CORES)),
        trace=trace,
        trace_cores=trace_cores,
    )
    full = np.concatenate([r["out"] for r in res.results], axis=0)
    return full, res


def kernel(x, y, L):
    full, _ = run_sharded(x, y, L)
    return full


# revision 6
# speedup vs baseline: 4.2669x; 1.1266x over previous
"""Low-rank Mahalanobis distance kernel for 8x TRN2 NeuronCores.

Full op: d2[i,j] = max(0, ||L(x_i - y_j)||^2) for x,y [8192,1024], L [128,1024].

Strategy:
  - Host precomputes the cheap projections xL = x@L.T, yL = y@L.T (~2% of
    total FLOPs) plus row norms, and lays everything out in the layouts the
    PE wants (rank on partitions). The -2 of the cross term is folded into
    the x projection on the host.
  - Rows of x are sharded 8 ways; each core computes a [1024, 8192] slice of
    the output. Per [128,1024] PSUM tile (2 banks): two K=128 bf16 matmuls
    give -2*cross; VectorE accumulates yn_j in-place from an SBUF broadcast
    plane (built once by GpSimd partition_broadcast from the f32 yn row);
    ScalarE writes Relu(psum + xn_i) into a [128, 8192] SBUF strip whose
    halves ship to HBM as 2MB DMAs.
  - The PE is kept to the 128 irreducible cross matmuls per core: sustained
    PE activity is clock-throttled to 1.2 GHz here, so rank-1 plane matmuls
    (which stream N columns just like a K=128 matmul) are deliberately off
    the PE; fp32 matmuls (HI/LO split + throttle) doubly so.
"""

import sys

sys.path.insert(0, "/opt/trn_rl_repo")

import ml_dtypes
import numpy as np

N = 8192  # rows of x == output rows
M = 8192  # rows of y == output cols
DIM = 1024
RANK = 128
N_CORES = 8
ROWS_PER_CORE = N // N_CORES  # 1024
IB = ROWS_PER_CORE // 128  # 8 i-blocks (strips) per core
JW = 512  # moving free dim per matmul (one PSUM bank of f32)
PTW = 1024  # psum tile width (2 banks) -> one epilogue op per 1024 cols
JT = M // PTW  # 8 psum tiles per strip
GRP = 4  # psum tiles in flight (4 x 2 banks = all of PSUM)
HALF = M // 2  # output DMA granularity (2MB half-strips)

BF16 = ml_dtypes.bfloat16

_CACHE = {}


def _build_nc():
    from contextlib import ExitStack

    import concourse.bacc as bacc
    import concourse.mybir as mybir
    import concourse.tile as tile

    dt = mybir.dt
    nc = bacc.Bacc("TRN2", target_bir_lowering=False, debug=False)

    xlt = nc.dram_tensor("xlt", [RANK, ROWS_PER_CORE], dt.bfloat16, kind="ExternalInput").ap()
    ylt = nc.dram_tensor("ylt", [RANK, M], dt.bfloat16, kind="ExternalInput").ap()
    xn = nc.dram_tensor("xn", [128, IB], dt.float32, kind="ExternalInput").ap()
    ynr = nc.dram_tensor("ynr", [1, M], dt.float32, kind="ExternalInput").ap()
    out = nc.dram_tensor("out", [ROWS_PER_CORE, M], dt.float32, kind="ExternalOutput").ap()

    with tile.TileContext(nc) as tc, ExitStack() as ctx:
        consts = ctx.enter_context(tc.tile_pool(name="consts", bufs=1))
        strips = ctx.enter_context(tc.tile_pool(name="strips", bufs=2))
        psum = ctx.enter_context(tc.tile_pool(name="psum", bufs=1, space="PSUM"))

        # small/early inputs first so the first matmuls start ASAP
        xlt_sb = consts.tile([RANK, ROWS_PER_CORE], dt.bfloat16)
        nc.sync.dma_start(xlt_sb[:], xlt[:])
        xn_sb = consts.tile([128, IB], dt.float32)
        nc.sync.dma_start(xn_sb[:], xn[:])
        ynr_sb = consts.tile([1, M], dt.float32)
        nc.sync.dma_start(ynr_sb[:], ynr[:])
        ylt_sb = consts.tile([RANK, M], dt.bfloat16)
        for ch in range(4):
            nc.sync.dma_start(
                ylt_sb[:, ch * (M // 4) : (ch + 1) * (M // 4)],
                ylt[:, ch * (M // 4) : (ch + 1) * (M // 4)],
            )
        # yn broadcast plane, built by GpSimd (otherwise idle), in chunks so
        # the first epilogues aren't gated on the whole 4MB
        ynb_sb = consts.tile([128, M], dt.float32)
        for ch in range(8):
            nc.gpsimd.partition_broadcast(
                ynb_sb[:, ch * PTW : (ch + 1) * PTW],
                ynr_sb[0:1, ch * PTW : (ch + 1) * PTW],
            )

        relu = mybir.ActivationFunctionType.Relu
        for ib in range(IB):
            strip = strips.tile([128, M], dt.float32, tag="strip")
            xlt_blk = xlt_sb[:, ib * 128 : (ib + 1) * 128]
            xn_col = xn_sb[:, ib : ib + 1]
            for g in range(JT // GRP):
                pts = [
                    psum.tile([128, PTW], dt.float32, tag=f"pt{k}", name=f"pt{k}")
                    for k in range(GRP)
                ]
                for k in range(GRP):
                    jt = g * GRP + k
                    for h in range(PTW // JW):
                        j0 = jt * PTW + h * JW
                        nc.tensor.matmul(
                            pts[k][:, h * JW : (h + 1) * JW],
                            lhsT=xlt_blk,
                            rhs=ylt_sb[:, j0 : j0 + JW],
                            start=True,
                            stop=True,
                        )
                for k in range(GRP):
                    jt = g * GRP + k
                    nc.vector.tensor_add(
                        pts[k][:], pts[k][:], ynb_sb[:, jt * PTW : (jt + 1) * PTW]
                    )
                for k in range(GRP):
                    jt = g * GRP + k
                    nc.scalar.activation(
                        strip[:, jt * PTW : (jt + 1) * PTW],
                        pts[k][:],
                        relu,
                        bias=xn_col,
                        scale=1.0,
                    )
                nc.sync.dma_start(
                    out[ib * 128 : (ib + 1) * 128, g * HALF : (g + 1) * HALF],
                    strip[:, g * HALF : (g + 1) * HALF],
                )

    nc.compile()
    return nc


def _prepare_in_maps(x, y, L):
    x = np.ascontiguousarray(x, dtype=np.float32)
    y = np.ascontiguousarray(y, dtype=np.float32)
    L = np.ascontiguousarray(L, dtype=np.float32)

    xL = x @ L.T  # [N, RANK]
    yL = y @ L.T  # [M, RANK]
    xn = np.einsum("ij,ij->i", xL, xL).astype(np.float32)  # [N]
    yn = np.einsum("ij,ij->i", yL, yL).astype(np.float32)  # [M]

    xLT = np.ascontiguousarray((-2.0 * xL).T.astype(BF16))  # [RANK, N]
    yLT = np.ascontiguousarray(yL.T.astype(BF16))  # [RANK, M]
    ynr = np.ascontiguousarray(yn.reshape(1, M))

    in_maps = []
    for c in range(N_CORES):
        r0 = c * ROWS_PER_CORE
        r1 = r0 + ROWS_PER_CORE
        # xn in [128 partitions, IB] column layout: col b holds xn of i-block b
        xn_cols = np.ascontiguousarray(xn[r0:r1].reshape(IB, 128).T)
        in_maps.append(
            {
                "xlt": np.ascontiguousarray(xLT[:, r0:r1]),
                "ylt": yLT,
                "xn": xn_cols,
                "ynr": ynr,
            }
        )
    return in_maps


def run_sharded(x, y, L, trace=False, trace_cores=None):
    """Run the device kernel; returns (full_output, BassKernelResults)."""
    from concourse.bass_utils import run_bass_kernel_spmd

    if "nc" not in _CACHE:
        _CACHE["nc"] = _build_nc()
    nc = _CACHE["nc"]

    in_maps = _prepare_in_maps(x, y, L)
    res = run_bass_kernel_spmd(
        nc,
        in_maps,
        list(range(N_CORES)),
        trace=trace,
        trace_cores=trace_cores,
    )
    full = np.concatenate([r["out"] for r in res.results], axis=0)
    return full, res


def kernel(x, y, L):
    full, _ = run_sharded(x, y, L)
    return full
